# revision 21
# baseline (speedup 1.0000x reference)
"""Trainium2 Bass kernel for the AudNet 4-layer LIF spiking network.

Reference computation (per time step t of 81, batch 4096):
    s1, m1 = lif(x_t @ w1.T + b1, m1)     # 129 -> 1000
    s2, m2 = lif(s1 @ w2.T + b2, m2)      # 1000 -> 1000
    s3, m3 = lif(s2 @ w3.T + b3, m3)      # 1000 -> 20
    s4, m4 = lif(s3 @ w4.T + b4, m4)      # 20 -> 10
with lif: reset = (m > 1); m' = 0.95*m + cur - reset; spk = (m' > 1)
Outputs: (s4, m4) per step -> each [81, 4096, 10].

Strategy (v2):
- Data parallel over 8 NeuronCores: 512 batch rows per core; weights
  replicated; no cross-device traffic.
- L1 (continuous x, hypersensitive to quantization -> kept exact):
  fp32r hi/lo x fp32r hi/lo, 3 cross products + K=5 combo matmul
  carrying feature 129 and the biases. 32 matmuls/step.
- L2 (binary spikes allow fp8): w2 = e4m3(w2) + rne12(w2 - e4m3(w2)).
  The e4m3 coarse term runs in DoubleRow mode (0.5 cyc/col, K=250 per
  matmul over two 125-row planes padded to 128 partitions); the fp32r
  fine term (residual |r|<2^-9.5 captured to 2^-22.5 absolute) runs at
  1 cyc/col. Both accumulate into the SAME psum bank at natural scale:
  no scale-fixup pass. ~17.5-bit weights; 12 output spike flips of
  3.3M on device (rel 7.9e-3 vs the 2e-2 gate).
  64 fp32r + 32 DR matmuls/step (was 128 fp32r).
- L3 batch-major: out[b, h] tiles [128, 20] so each matmul is charged
  N=20 cols instead of 512 (the cost model bills matmuls by output free
  size). lhsT = s2 spike slices (bf16, exact), rhs = bf16 w3 (8-bit; 0
  extra flips measured vs 16-bit on CPU sim). Bias rides a ones row
  (partition 125 of the last k-tile). Spikes transposed back to
  hidden-major [20, 512] via 4 bf16 PE transposes for L4.
- L4: single fp32r matmul, w4/b4 rne12 (mem rel err 1.5e-4).
- Biases ride the contraction dim everywhere (ones rows).
- LIF per layer per step: opA m += psum (DVE), opB spk = m > 1 (DVE),
  opC m = beta*m - spk (gpsimd 2-op for L1/L2 to keep DVE under the
  PE's ~25us/step); s1->e4m3 convert and s2->bf16 copy on ACT.
"""

import os
import sys

import numpy as np

for _p in ("/opt/trn_rl_repo", "/root/.axon_site/_ro/trn_rl_repo"):
    if os.path.isdir(_p) and _p not in sys.path:
        sys.path.insert(0, _p)

import concourse.bacc as bacc
import concourse.bass as bass
import concourse.mybir as mybir
import concourse.tile as tile
from concourse.bass_utils import run_bass_kernel_spmd
from concourse.tile_rust import add_dep_helper

# Problem constants (hardcoded; kernel.py must be self-contained).
T = 81          # time steps
F = 129         # input features per step
H = 1000        # hidden units (layers 1, 2)
HT = 125        # hidden tile rows  (H = 8 * 125)
NH = 8          # number of hidden tiles
H3 = 20         # layer-3 units
H4 = 10         # output units
BATCH = 4096
NCORES = 8
B = BATCH // NCORES   # 512 batch rows per core
BETA = 0.95
THRESH = 1.0
XR = 2 * 128 + 5      # x_aug rows: xh[0:128], xl[0:128], 5 combo rows

F32 = mybir.dt.float32
F32R = mybir.dt.float32r
E4 = mybir.dt.float8e4
BF16 = mybir.dt.bfloat16
AOP = mybir.AluOpType
DR = mybir.MatmulPerfMode.DoubleRow


def build_bass():
    # Bacc (not raw Bass): its compile() runs generate_event_semaphores /
    # move_matmul_waits_to_ldweights, required because TRN2 Matmult
    # instructions can carry at most one sync wait.
    nc = bacc.Bacc(trn_type="TRN2", target_bir_lowering=False)

    x_d = nc.dram_tensor("x_aug", [T, XR, B], F32R, kind="ExternalInput")
    w1h_d = nc.dram_tensor("w1h", [128, H], F32R, kind="ExternalInput")
    w1l_d = nc.dram_tensor("w1l", [128, H], F32R, kind="ExternalInput")
    w1c_d = nc.dram_tensor("w1c", [5, H], F32R, kind="ExternalInput")
    # L2 fine term: rne12(w2 - e4m3(w2)), bias residual on k-tile 7 row 125
    w2f_d = nc.dram_tensor("w2f", [NH, HT + 1, H], F32R, kind="ExternalInput")
    # L2 coarse term e4m3 for DoubleRow: [p, kd, plane, h*128+m],
    # contraction k = (2*kd+plane)*125 + p (p<125); row 125 of (3,1) = bias
    w2c_d = nc.dram_tensor("w2c", [128, 4, 2, NH * 128], E4,
                           kind="ExternalInput")
    # L3 single-term bf16, batch-major rhs: [kt, p, h3]
    w3r_d = nc.dram_tensor("w3r", [NH, HT + 1, H3], BF16,
                           kind="ExternalInput")
    # L4 single fp32r: rows 0..19 = rne12(w4.T), row 20 = rne12(b4)
    w4r_d = nc.dram_tensor("w4r", [H3 + 1, H4], F32R, kind="ExternalInput")
    iden_d = nc.dram_tensor("iden", [128, 128], BF16, kind="ExternalInput")
    outs_d = nc.dram_tensor("out_s", [T, H4, B], F32, kind="ExternalOutput")
    outm_d = nc.dram_tensor("out_m", [T, H4, B], F32, kind="ExternalOutput")

    with tile.TileContext(nc) as tc:
        with (
            tc.tile_pool(name="pers", bufs=1) as pers,
            tc.tile_pool(name="xpool", bufs=3) as xpool,
            tc.tile_pool(name="ps1", bufs=2, space="PSUM") as ps1,
            tc.tile_pool(name="ps2", bufs=3, space="PSUM") as ps2,
            tc.tile_pool(name="ps3", bufs=1, space="PSUM") as ps3,
            tc.tile_pool(name="ps3t", bufs=1, space="PSUM") as ps3t,
            tc.tile_pool(name="ps4", bufs=1, space="PSUM") as ps4,
        ):
            # ---- persistent SBUF tensors ----
            w1h = pers.tile([128, H], F32R, tag="w1h")
            w1l = pers.tile([128, H], F32R, tag="w1l")
            w1c = pers.tile([5, H], F32R, tag="w1c")
            w2f = pers.tile([HT + 1, NH * H], F32R, tag="w2f")   # [126, 8000]
            w2c = pers.tile([128, 4, 2, NH * 128], E4, tag="w2c")
            w3r = pers.tile([HT + 1, NH * H3], BF16, tag="w3r")
            w4r = pers.tile([H3 + 1, H4], F32R, tag="w4r")
            iden = pers.tile([128, 128], BF16, tag="iden")
            m1 = pers.tile([HT, NH * B], F32, tag="m1")          # [125, 4096]
            m2 = pers.tile([HT, NH * B], F32, tag="m2")
            m3b = pers.tile([128, 4 * H3], F32, tag="m3b")       # batch-major
            m4 = pers.tile([H4, B], F32, tag="m4")               # [10, 512]
            s1 = pers.tile([HT + 1, NH * B], F32R, tag="s1")     # [126, 4096]
            s1e4 = pers.tile([128, NH, B], E4, tag="s1e4")       # DR rhs
            s2 = pers.tile([HT + 1, NH * B], F32R, tag="s2")
            s2b = pers.tile([HT + 1, NH * B], BF16, tag="s2b")
            s3b = pers.tile([128, 4 * H3], BF16, tag="s3b")      # batch-major
            s3 = pers.tile([H3 + 1, B], F32R, tag="s3")          # [21, 512]
            s4 = pers.tile([H4, B], F32, tag="s4")

            # fp32 views of the fp32r tiles for elementwise producers
            s1f = s1[:].bitcast(F32)
            s2f = s2[:].bitcast(F32)
            s3f = s3[:].bitcast(F32)

            def load_x(t, eng=None):
                eng = eng or nc.sync
                xh = xpool.tile([128, B], F32R, tag="xh", name="xh")
                xl = xpool.tile([128, B], F32R, tag="xl", name="xl")
                xc = xpool.tile([5, B], F32R, tag="xc", name="xc")
                eng.dma_start(xh[:], x_d[t, 0:128, :])
                eng.dma_start(xl[:], x_d[t, 128:256, :])
                eng.dma_start(xc[:], x_d[t, 256:261, :])
                return xh, xl, xc

            # layer-1 weights + x(0) first (they gate step 0), then the bulk
            w1dmas = []
            for sb, dr_ in [(w1h, w1h_d), (w1l, w1l_d), (w1c, w1c_d)]:
                w1dmas.append(nc.sync.dma_start(sb[:], dr_[:]))
            x0 = load_x(0)
            # w2f/w2c gate step-0 L2 -> issue them first; w3r/w4r/iden are
            # needed ~17us later and absorb on the first L3 matmul instead
            wdmas = []
            for k in range(NH):
                eng = nc.sync if k % 2 == 0 else nc.scalar
                wdmas.append(eng.dma_start(
                    w2f[:, k * H:(k + 1) * H], w2f_d[k]))
            wdmas.append(nc.scalar.dma_start(w2c[:], w2c_d[:]))
            wdmas3 = [nc.scalar.dma_start(w4r[:], w4r_d[:]),
                      nc.scalar.dma_start(iden[:], iden_d[:])]
            for tk in range(NH):
                wdmas3.append(nc.scalar.dma_start(
                    w3r[:, tk * H3:(tk + 1) * H3], w3r_d[tk]))

            # Matmult instructions can carry at most ONE sync wait in the
            # TRN2 ISA, so have PE nops absorb the weight-DMA waits before
            # any matmul.
            def absorb(dmas):
                nops = []
                for d in dmas:
                    nop = nc.tensor.nop(nofuse=True)
                    add_dep_helper(nop.ins, d.ins, sync=True,
                                   reason="absorb weight-DMA wait on PE")
                    nops.append(nop)
                return nops

            absorbers = absorb(w1dmas)

            # ---- state init ----
            nc.vector.memset(m1[:], 0.0)
            nc.vector.memset(m2[:], 0.0)
            nc.gpsimd.memset(m3b[:], 0.0)
            nc.gpsimd.memset(m4[:], 0.0)
            # ones rows feeding the bias fold. Engine ops need partition
            # bases in {0,32,64,96}, so memset a wider aligned region; rows
            # 96..124 are overwritten by per-step spike writes before any
            # matmul reads them.
            nc.vector.memset(s1f[96:HT + 1, (NH - 1) * B:], 1.0)
            nc.vector.memset(s2f[96:HT + 1, (NH - 1) * B:], 1.0)
            nc.vector.memset(s2b[96:HT + 1, (NH - 1) * B:], 1.0)
            nc.gpsimd.memset(s3f[:, :], 1.0)   # row 20 stays the ones row
            # s1e4: zero everywhere (incl. pad partitions 125..127), then
            # ones on the bias plane (kt 7); rows 96..124 rewritten per step,
            # rows 126/127 stay 1.0 but multiply zero weight columns.
            nc.gpsimd.memset(s1e4[:, :, :], 0.0)
            nc.gpsimd.memset(s1e4[96:128, NH - 1, :], 1.0)

            def l1_block(xh, xl, xc):
                """Layer-1 psums + LIF opA/opB for one step, per hidden tile."""
                first_mm = None
                for h in range(NH):
                    p1 = ps1.tile([HT, B], F32, tag="p1")
                    c0 = h * HT
                    mm = nc.tensor.matmul(p1[:], w1h[:, c0:c0 + HT], xh[:],
                                          start=True, stop=False)
                    if first_mm is None:
                        first_mm = mm
                    nc.tensor.matmul(p1[:], w1h[:, c0:c0 + HT], xl[:],
                                     start=False, stop=False)
                    nc.tensor.matmul(p1[:], w1l[:, c0:c0 + HT], xh[:],
                                     start=False, stop=False)
                    nc.tensor.matmul(p1[:], w1c[:, c0:c0 + HT], xc[:],
                                     start=False, stop=True)
                    cols = slice(h * B, (h + 1) * B)
                    nc.vector.tensor_tensor(m1[:, cols], p1[:], m1[:, cols],
                                            AOP.add)
                    nc.vector.tensor_scalar(s1[0:HT, cols], m1[:, cols],
                                            THRESH, None, AOP.is_gt)
                    # s1 -> e4m3 DR-rhs plane for this k-tile (ACT);
                    # per-tile so the next step's DR matmuls aren't gated
                    # on a monolithic convert
                    nc.scalar.copy(s1e4[0:HT, h, :], s1f[0:HT, cols])
                return first_mm

            def l1_state_update():
                # m1 = beta*m1 - spk1; single DVE pass — gpsimd runs its
                # Add/Multiply at 0.42 efficiency and saturates when it
                # carries both layers' updates, stalling the L1 psum-bank
                # recycle chain
                nc.vector.scalar_tensor_tensor(m1[:], m1[:], BETA,
                                               s1f[0:HT, :],
                                               AOP.mult, AOP.subtract)

            # ---- prologue: step 0 layer-1 ----
            first_mm = l1_block(*x0)
            for nop in absorbers:
                add_dep_helper(first_mm.ins, nop.ins, sync=False,
                               reason="keep absorbers before first matmul")
            l1_state_update()

            late_absorbers = absorb(wdmas)
            l3_absorbers = absorb(wdmas3)

            def l4_block(t):
                """Layer 4 for step t + LIF + output DMAs (all elementwise on
                gpsimd: DVE is the secondary bottleneck)."""
                p4 = ps4.tile([H4, B], F32, tag="p4")
                nc.tensor.matmul(p4[:], w4r[:], s3[:], start=True, stop=True)
                nc.vector.tensor_tensor(m4[:], p4[:], m4[:], AOP.add)
                nc.sync.dma_start(outm_d[t], m4[:])
                nc.gpsimd.tensor_scalar(s4[:], m4[:], THRESH, None, AOP.is_gt)
                nc.sync.dma_start(outs_d[t], s4[:])
                nc.vector.scalar_tensor_tensor(m4[:], m4[:], BETA, s4[:],
                                               AOP.mult, AOP.subtract)

            # ---- main loop over steps ----
            for i in range(T):
                if i < T - 1:
                    xh, xl, xc = load_x(i + 1)

                # layer 2 of step i: per h-tile, 8 fp32r fine k-tiles + 4
                # e4m3 DoubleRow coarse k-tiles into one psum bank
                for h in range(NH):
                    p2 = ps2.tile([128, B], F32, tag="p2")
                    c0 = h * HT
                    for k in range(NH):
                        kk = HT + 1 if k == NH - 1 else HT
                        mm2 = nc.tensor.matmul(
                            p2[0:HT, :],
                            w2f[0:kk, k * H + c0:k * H + c0 + HT],
                            s1[0:kk, k * B:(k + 1) * B],
                            start=(k == 0), stop=False,
                            skip_group_check=True)
                        if i == 0 and h == 0 and k == 0:
                            for nop in late_absorbers:
                                add_dep_helper(
                                    mm2.ins, nop.ins, sync=False,
                                    reason="absorbers before first L2 mm")
                    for kd in range(4):
                        nc.tensor.matmul(
                            p2[:],
                            w2c[:, kd, :, h * 128:(h + 1) * 128],
                            s1e4[:, 2 * kd:2 * kd + 2, :],
                            start=False, stop=(kd == 3),
                            perf_mode=DR, skip_group_check=True)
                    cols = slice(h * B, (h + 1) * B)
                    nc.vector.tensor_tensor(m2[:, cols], p2[0:HT, :],
                                            m2[:, cols], AOP.add)
                    nc.vector.tensor_scalar(s2[0:HT, cols], m2[:, cols],
                                            THRESH, None, AOP.is_gt)
                    if i == T - 1:
                        # last step: per-tile s2b copy so the epilogue chain
                        # (s2b -> L3 -> transpose -> L4) starts 7 tiles early
                        nc.scalar.copy(s2b[0:HT, cols], s2f[0:HT, cols])
                    # m2 state update per tile (gpsimd), right after the
                    # threshold so it never gates the next step's opA
                    nc.gpsimd.tensor_scalar_mul(m2[:, cols], m2[:, cols],
                                                BETA)
                    nc.gpsimd.tensor_tensor(m2[:, cols], m2[:, cols],
                                            s2f[0:HT, cols], AOP.subtract)

                # layer-1 psums + LIF for step i+1
                if i < T - 1:
                    l1_block(xh, xl, xc)

                # layer 4 of step i-1 (deferred so spk3 is long ready)
                if i > 0:
                    l4_block(i - 1)

                # s2 -> bf16 for the L3 lhsT (ACT, off critical path)
                if i < T - 1:
                    nc.scalar.copy(s2b[0:HT, :], s2f[0:HT, :])

                # layer 3 of step i, batch-major: out tiles [128 batch, 20]
                p3 = ps3.tile([128, 4 * H3], F32, tag="p3")
                for bb in range(4):
                    for k in range(NH):
                        kk = HT + 1 if k == NH - 1 else HT
                        mm3 = nc.tensor.matmul(
                            p3[:, bb * H3:(bb + 1) * H3],
                            s2b[0:kk, k * B + bb * 128:k * B + (bb + 1) * 128],
                            w3r[0:kk, k * H3:(k + 1) * H3],
                            start=(k == 0),
                            stop=(k == NH - 1))
                        if i == 0 and bb == 0 and k == 0:
                            for nop in l3_absorbers:
                                add_dep_helper(
                                    mm3.ins, nop.ins, sync=False,
                                    reason="absorbers before first L3 mm")
                nc.vector.tensor_tensor(m3b[:], p3[:], m3b[:], AOP.add)
                nc.vector.tensor_scalar(s3b[:], m3b[:], THRESH, None,
                                        AOP.is_gt)
                nc.vector.scalar_tensor_tensor(m3b[:], m3b[:], BETA, s3b[:],
                                               AOP.mult, AOP.subtract)
                # transpose spikes back to hidden-major [20, 512] for L4
                pt3 = ps3t.tile([H3, B], BF16, tag="pt3")
                for bb in range(4):
                    nc.tensor.transpose(pt3[:, bb * 128:(bb + 1) * 128],
                                        s3b[:, bb * H3:(bb + 1) * H3],
                                        iden[:])
                nc.scalar.copy(s3[0:H3, :], pt3[:])

                # layer-1 state update for step i+1
                if i < T - 1:
                    l1_state_update()

            # ---- epilogue ----
            l4_block(T - 1)

    nc.compile()
    return nc


_CACHE = {}


def _get_nc():
    if "nc" not in _CACHE:
        _CACHE["nc"] = build_bass()
    return _CACHE["nc"]


def _rne12(a):
    """Round fp32 to 12 significand bits (the fp32r grid), RNE —
    bit-identical to the device's fp32r rounding."""
    drop = np.uint64(12)
    u = np.ascontiguousarray(a, np.float32).view(np.uint32).astype(np.uint64)
    half = np.uint64(1 << 11)
    lsb = (u >> drop) & np.uint64(1)
    u2 = ((u + half - np.uint64(1) + lsb) >> drop << drop)
    return u2.astype(np.uint32).view(np.float32).reshape(a.shape)


def _hilo(a):
    hi = _rne12(a)
    lo = _rne12(np.asarray(a, np.float32) - hi)
    return hi, lo


def _prep_inputs(x, w1, b1, w2, b2, w3, b3, w4, b4):
    import ml_dtypes
    bf16 = ml_dtypes.bfloat16
    e4np = ml_dtypes.float8_e4m3

    x = np.ascontiguousarray(x, np.float32)
    # xs[t, f, b_global]; step t of the reference reads x[:, f*T + t]
    xt = np.ascontiguousarray(
        np.transpose(x.reshape(BATCH, F, T), (2, 1, 0)))   # [T, F, BATCH]
    xth, xtl = _hilo(xt)

    w1T = np.ascontiguousarray(w1.T.astype(np.float32))    # [129, 1000]
    w1h, w1l = _hilo(w1T[:128])
    whL, wlL = _hilo(w1T[128])
    b1h, b1l = _hilo(b1.astype(np.float32))
    w1c = np.stack([whL, whL, wlL, b1h, b1l])              # [5, 1000]

    # ---- L2: e4m3 coarse + fp32r fine residual ----
    w2T = np.ascontiguousarray(w2.T.astype(np.float32))    # [1000, 1000]
    w2coarse = w2T.astype(e4np)
    w2fine = _rne12(w2T - w2coarse.astype(np.float32))
    b2c = b2.astype(np.float32).astype(e4np)
    b2fine = _rne12(b2.astype(np.float32) - b2c.astype(np.float32))

    w2f = np.zeros((NH, HT + 1, H), np.float32)
    for k in range(NH):
        w2f[k, :HT] = w2fine[k * HT:(k + 1) * HT]
    w2f[NH - 1, HT] = b2fine

    w2c = np.zeros((128, 4, 2, NH * 128), e4np)
    wc = w2coarse.reshape(NH, HT, H)                       # [kt, p, h1out]
    for kd in range(4):
        for j in range(2):
            kt = 2 * kd + j
            for h in range(NH):
                w2c[0:HT, kd, j, h * 128:h * 128 + HT] = \
                    wc[kt, :, h * HT:(h + 1) * HT]
    for h in range(NH):
        w2c[HT, 3, 1, h * 128:h * 128 + HT] = b2c[h * HT:(h + 1) * HT]

    # ---- L3: single bf16 term, batch-major rhs ----
    w3T = w3.T.astype(np.float32).astype(bf16)             # [1000, 20]
    b3q = b3.astype(np.float32).astype(bf16)
    w3r = np.zeros((NH, HT + 1, H3), bf16)
    for k in range(NH):
        w3r[k, :HT] = w3T[k * HT:(k + 1) * HT]
    w3r[NH - 1, HT] = b3q

    # ---- L4: single fp32r ----
    w4r = np.zeros((H3 + 1, H4), np.float32)
    w4r[0:H3] = _rne12(w4.T.astype(np.float32))
    w4r[H3] = _rne12(b4.astype(np.float32))

    iden = np.eye(128, dtype=bf16)

    in_maps = []
    for c in range(NCORES):
        xc = np.empty((T, XR, B), np.float32)
        xc[:, 0:128, :] = xth[:, 0:128, c * B:(c + 1) * B]
        xc[:, 128:256, :] = xtl[:, 0:128, c * B:(c + 1) * B]
        xc[:, 256, :] = xth[:, 128, c * B:(c + 1) * B]
        xc[:, 257, :] = xtl[:, 128, c * B:(c + 1) * B]
        xc[:, 258, :] = xth[:, 128, c * B:(c + 1) * B]
        xc[:, 259, :] = 1.0
        xc[:, 260, :] = 1.0
        in_maps.append({
            "x_aug": xc, "w1h": w1h, "w1l": w1l, "w1c": w1c,
            "w2f": w2f, "w2c": w2c, "w3r": w3r, "w4r": w4r, "iden": iden,
        })
    return in_maps


def _gather(results):
    spk = np.concatenate(
        [np.transpose(r["out_s"], (0, 2, 1)) for r in results], axis=1)
    mem = np.concatenate(
        [np.transpose(r["out_m"], (0, 2, 1)) for r in results], axis=1)
    return spk, mem


def kernel(x, w1, b1, w2, b2, w3, b3, w4, b4, _trace=False, _trace_kwargs=None):
    # accept numpy or jax arrays, any float dtype
    x, w1, b1, w2, b2, w3, b3, w4, b4 = (
        np.asarray(a, dtype=np.float32)
        for a in (x, w1, b1, w2, b2, w3, b3, w4, b4))
    nc = _get_nc()
    in_maps = _prep_inputs(x, w1, b1, w2, b2, w3, b3, w4, b4)
    res = run_bass_kernel_spmd(
        nc, in_maps, core_ids=list(range(NCORES)),
        trace=_trace, **(_trace_kwargs or {}))
    out = _gather(res.results)
    if _trace:
        return out, res
    return out
